# revision 19
# baseline (speedup 1.0000x reference)
"""Distributed 2-layer GCN (PyG GCNConv-style) on 8 Trainium2 NeuronCores.

Strategy (hardcoded for N=100000, E=3.2M, 512->256->128->4):
  - Nodes are degree-balanced into (ncores*W) windows of 128 nodes; window b is
    owned by core (b % ncores). A node's "global position" is its row in the
    AllGathered feature table, so gathers use plain int positions.
  - Per layer: local dense matmul (bf16 on PE, fp32 PSUM), rows pre-scaled by
    dinv, results AllGathered to 4 replicated contiguous bf16 class tables in
    DRAM (class = slot//32; int16 gather indices fit because each class table
    has NPAD/4 < 32768 rows; contiguous rows gather ~13% faster than strided).
  - Aggregation: windows are processed in groups of GRP=4. Per (group, class)
    ONE large dma_gather (custom SWDGE gather, int16 indices, ~4-5k rows per
    call, single_packet=False — single_packet hangs the DMA above 1024 rows)
    pulls all the group's dst-sorted edge source rows into SBUF; per 128-edge
    tile a one-hot S matrix built on DVE (is_equal vs iota, all-bf16 for 2x
    rate) feeds a PE matmul S^T @ msgs that segment-sums into the window's
    PSUM accumulator (one PSUM bank per window: the PE keeps GRP accumulation
    groups open at once, and PSUM allows one open group per 2KB bank).
    Padding slots carry dcol=128 which never matches iota -> contribute 0.
    The gather is descriptor-bound (~9ns/row aggregate, independent of row
    bytes): per-edge HBM random reads are the kernel's floor.
  - Epilogue: z = dinv*acc + b; relu; layer 2 repeats; final logits + log
    softmax (batched Ln to avoid ACT table thrash).
"""
import math
import numpy as np

import concourse.bass as bass
import concourse.mybir as mybir
import concourse.bass_utils as bass_utils
from concourse import bacc, tile
from concourse.bass_interp import get_hw_module

P = 128
F32 = mybir.dt.float32
BF16 = mybir.dt.bfloat16
I16 = mybir.dt.int16


class Cfg:
    def __init__(self, N, F_IN, H1, H2, C, ncores=8, W=None, grp=4):
        self.N, self.F_IN, self.H1, self.H2, self.C = N, F_IN, H1, H2, C
        self.ncores = ncores
        Bc = ncores * P
        self.W = W if W is not None else math.ceil(N / Bc)
        self.NPAD = self.W * Bc
        assert self.NPAD >= N and self.NPAD % 4 == 0
        assert self.NPAD // 4 <= 32768
        self.KI = F_IN // P
        self.K2 = H1 // P
        self.B = self.W * P  # nodes per core
        self.GRP = grp
        self.NG = math.ceil(self.W / grp)


FULL = Cfg(N=100000, F_IN=512, H1=256, H2=128, C=4)


# ---------------------------------------------------------------- host side
def preprocess(cfg, x, edge_index, W1, b1, W2, b2, Wl, bl):
    N, NC, W, NPAD, B = cfg.N, cfg.ncores, cfg.W, cfg.NPAD, cfg.B
    GRP, NG = cfg.GRP, cfg.NG
    NW = NC * W

    src = np.asarray(edge_index[0], dtype=np.int64)
    dst = np.asarray(edge_index[1], dtype=np.int64)
    deg = np.bincount(dst, minlength=N).astype(np.float64) + 1.0
    dinv = (1.0 / np.sqrt(deg)).astype(np.float32)

    # node -> global position, degree-balanced across windows (snake fill)
    degall = np.zeros(NPAD, np.int64)
    degall[:N] = deg.astype(np.int64)
    order = np.argsort(-degall, kind="stable")
    i = np.arange(NPAD)
    phase = i % (2 * NW)
    binid = np.where(phase < NW, phase, 2 * NW - 1 - phase)
    by_bin = np.argsort(binid, kind="stable")
    slot = np.empty(NPAD, np.int64)
    slot[by_bin] = i % P  # within each bin, slots fill 0..127 in arrival order
    # position: core = bin % NC, window = bin // NC
    core_of_bin = binid % NC
    w_of_bin = binid // NC
    g_of_i = core_of_bin * B + w_of_bin * P + slot
    pos = np.empty(NPAD, np.int64)
    pos[order] = g_of_i

    node_at = np.empty(NPAD, np.int64)
    node_at[pos] = np.arange(NPAD)

    # edge list incl. one self edge per real node
    S_pos = np.concatenate([pos[src], pos[np.arange(N)]])
    D_pos = np.concatenate([pos[dst], pos[np.arange(N)]])
    core_e = D_pos // B
    w_e = (D_pos % B) // P
    dcol_e = (D_pos % P).astype(np.float32)
    # class tables: class = slot//32; row in class table = rank*B4 + w*32 + slot%32
    B4 = B // 4
    s_slot = S_pos % P
    cls_e = s_slot // 32
    idx16_e = ((S_pos // B) * B4 + ((S_pos % B) // P) * 32
               + (s_slot % 32)).astype(np.int16)
    g_e = w_e // GRP
    wl_e = w_e % GRP

    key = (((core_e * NG + g_e) * 4 + cls_e) * GRP + wl_e).astype(np.int64)
    ordE = np.argsort(key, kind="stable")
    counts = np.bincount(key, minlength=NC * NG * 4 * GRP).reshape(NC, NG, 4, GRP)
    Tw = np.ceil(counts / P).astype(np.int64).max(axis=0)  # [NG, 4, GRP]
    TT = int(Tw.sum())
    SLOT = TT * P

    # tile offsets in global tile order (g, cl, wl)
    tile_off = np.zeros((NG, 4, GRP), np.int64)
    acc = 0
    for g in range(NG):
        for cl in range(4):
            for wl in range(GRP):
                tile_off[g, cl, wl] = acc
                acc += int(Tw[g, cl, wl])
    call_off = tile_off[:, :, 0]                    # [NG, 4]
    call_nt = Tw.sum(axis=2)                        # [NG, 4]

    starts = np.zeros(NC * NG * 4 * GRP + 1, np.int64)
    np.cumsum(counts.reshape(-1), out=starts[1:])

    idx16 = np.zeros((NC, SLOT), np.int16)
    dcol = np.full((NC, SLOT), float(P), np.float32)
    for c in range(NC):
        for g in range(NG):
            for cl in range(4):
                for wl in range(GRP):
                    k = ((c * NG + g) * 4 + cl) * GRP + wl
                    s0, s1 = starts[k], starts[k + 1]
                    n = s1 - s0
                    if n == 0:
                        continue
                    off = int(tile_off[g, cl, wl]) * P
                    seg = ordE[s0:s1]
                    idx16[c, off:off + n] = idx16_e[seg]
                    dcol[c, off:off + n] = dcol_e[seg]

    # wrap idx16 per gather call: element i of a call at [i%16, i//16], rep x8
    idx_w = np.zeros((NC, 16, SLOT // 16), np.int16)
    for g in range(NG):
        for cl in range(4):
            off = int(call_off[g, cl]) * P
            n = int(call_nt[g, cl]) * P
            if n == 0:
                continue
            blk = idx16[:, off:off + n].reshape(NC, n // 16, 16)
            idx_w[:, :, off // 16:(off + n) // 16] = blk.transpose(0, 2, 1)
    idx_rep = np.tile(idx_w, (1, 8, 1))  # [NC, 128, SLOT//16]

    dcol_t = dcol.reshape(NC, TT, P).transpose(0, 2, 1).copy()  # [NC, 128, TT]

    # x shard, transposed chunk layout: xt4[p, w, c2, m] = x[node(w*128+m), c2*128+p]
    xpad = np.zeros((NPAD, cfg.F_IN), np.float32)
    xpad[:N] = np.asarray(x, np.float32)
    dinvpad = np.ones(NPAD, np.float32)
    dinvpad[:N] = dinv

    xt4 = np.empty((NC, P, W, cfg.KI, P), np.float32)
    dinvl = np.empty((NC, P, W), np.float32)
    for c in range(NC):
        ids = node_at[c * B:(c + 1) * B]
        xl = xpad[ids]  # [B, F_IN]
        xt4[c] = xl.reshape(W, P, cfg.KI, P).transpose(3, 0, 2, 1)
        dinvl[c] = dinvpad[ids].reshape(W, P).T

    w1d = np.asarray(W1, np.float32).reshape(cfg.KI, P, cfg.H1).transpose(1, 0, 2)
    w2d = np.asarray(W2, np.float32).reshape(cfg.K2, P, cfg.H2).transpose(1, 0, 2)
    wld = np.asarray(Wl, np.float32)  # [H2=128, C]
    b1b = np.broadcast_to(np.asarray(b1, np.float32), (P, cfg.H1)).copy()
    b2b = np.broadcast_to(np.asarray(b2, np.float32), (P, cfg.H2)).copy()
    blb = np.broadcast_to(np.asarray(bl, np.float32), (P, cfg.C)).copy()
    iota = np.broadcast_to(np.arange(P, dtype=np.float32), (P, P)).copy()
    ident = np.eye(P, dtype=np.float32)

    import ml_dtypes
    bf = lambda a: np.asarray(a, np.float32).astype(ml_dtypes.bfloat16)

    in_maps = []
    for c in range(NC):
        in_maps.append({
            "xt4": bf(xt4[c]),
            "w1d": bf(w1d), "w2d": bf(w2d), "wld": bf(wld),
            "b1b": b1b, "b2b": b2b, "blb": blb,
            "dinvl": dinvl[c],
            "idx16": idx_rep[c],
            "dcol": bf(dcol_t[c]),
            "iota": bf(iota),
            "ident": bf(ident),
        })

    meta = dict(Tw=Tw, call_off=call_off, call_nt=call_nt, tile_off=tile_off,
                pos=pos, node_at=node_at, SLOT=SLOT, TT=TT)
    return in_maps, meta


def assemble_output(cfg, meta, results):
    N, NC, W, C, B = cfg.N, cfg.ncores, cfg.W, cfg.C, cfg.B
    rows = []
    for c in range(NC):
        r = results[c]["outst"].reshape(P, W, C).transpose(1, 0, 2).reshape(B, C)
        rows.append(r)
    allrows = np.concatenate(rows, axis=0)  # [NPAD, C] in position order
    return allrows[meta["pos"][:N]].astype(np.float32)


# ---------------------------------------------------------------- device side
def build_kernel(cfg, meta, upto="full"):
    NC, W, NPAD, B = cfg.ncores, cfg.W, cfg.NPAD, cfg.B
    H1, H2, C, KI, K2 = cfg.H1, cfg.H2, cfg.C, cfg.KI, cfg.K2
    GRP, NG = cfg.GRP, cfg.NG
    Tw = np.asarray(meta["Tw"])
    call_off = np.asarray(meta["call_off"])
    call_nt = np.asarray(meta["call_nt"])
    tile_off = np.asarray(meta["tile_off"])
    TT = int(Tw.sum())
    SLOT = TT * P
    NTMAX = int(call_nt.max())

    nc = bacc.Bacc("TRN2", target_bir_lowering=False, debug=False, num_devices=NC)

    xt4 = nc.dram_tensor("xt4", [P, W, KI, P], BF16, kind="ExternalInput")
    w1d = nc.dram_tensor("w1d", [P, KI, H1], BF16, kind="ExternalInput")
    w2d = nc.dram_tensor("w2d", [P, K2, H2], BF16, kind="ExternalInput")
    wld = nc.dram_tensor("wld", [P, C], BF16, kind="ExternalInput")
    b1b = nc.dram_tensor("b1b", [P, H1], F32, kind="ExternalInput")
    b2b = nc.dram_tensor("b2b", [P, H2], F32, kind="ExternalInput")
    blb = nc.dram_tensor("blb", [P, C], F32, kind="ExternalInput")
    dinvl = nc.dram_tensor("dinvl", [P, W], F32, kind="ExternalInput")
    idx16 = nc.dram_tensor("idx16", [P, SLOT // 16], I16, kind="ExternalInput")
    dcol = nc.dram_tensor("dcol", [P, TT], BF16, kind="ExternalInput")
    iota = nc.dram_tensor("iota", [P, P], BF16, kind="ExternalInput")
    ident = nc.dram_tensor("ident", [P, P], BF16, kind="ExternalInput")
    outst = nc.dram_tensor("outst", [P, W * C], F32, kind="ExternalOutput")

    rg = [list(range(NC))]

    def first_last_cl(g, wl):
        cls = [cl for cl in range(4) if Tw[g, cl, wl] > 0]
        return cls[0], cls[-1]

    with tile.TileContext(nc) as tc:
        with tc.tile_pool(name="const", bufs=1) as cpool, \
             tc.tile_pool(name="dram", bufs=1, space="DRAM") as dram:
            ag1_in = [dram.tile([B // 4, H1], BF16, name=f"ag1i{i}")
                      for i in range(4)]
            ag1_out = [dram.tile([NPAD // 4, H1], BF16, addr_space="Shared",
                                 name=f"ag1o{i}") for i in range(4)]
            ag2_in = [dram.tile([B // 4, H2], BF16, name=f"ag2i{i}")
                      for i in range(4)]
            ag2_out = [dram.tile([NPAD // 4, H2], BF16, addr_space="Shared",
                                 name=f"ag2o{i}") for i in range(4)]

            w1sb = cpool.tile([P, KI, H1], BF16)
            nc.sync.dma_start(w1sb[:], w1d[:])
            w2sb = cpool.tile([P, K2, H2], BF16)
            nc.sync.dma_start(w2sb[:], w2d[:])
            wlsb = cpool.tile([P, C], BF16)
            nc.sync.dma_start(wlsb[:], wld[:])
            b1sb = cpool.tile([P, H1], F32)
            nc.sync.dma_start(b1sb[:], b1b[:])
            b2sb = cpool.tile([P, H2], F32)
            nc.sync.dma_start(b2sb[:], b2b[:])
            blsb = cpool.tile([P, C], F32)
            nc.sync.dma_start(blsb[:], blb[:])
            dinvsb = cpool.tile([P, W], F32)
            nc.sync.dma_start(dinvsb[:], dinvl[:])
            iotasb = cpool.tile([P, P], BF16)
            nc.sync.dma_start(iotasb[:], iota[:])
            idsb = cpool.tile([P, P], BF16)
            nc.sync.dma_start(idsb[:], ident[:])
            dcolsb = cpool.tile([P, TT], BF16)
            nc.sync.dma_start(dcolsb[:], dcol[:])
            idxsb = cpool.tile([P, SLOT // 16], I16)
            nc.sync.dma_start(idxsb[:], idx16[:])

            lgst = cpool.tile([P, W * C], F32)
            sst = cpool.tile([P, W], F32)
            outsb = cpool.tile([P, W * C], F32)

            # ---------------- phase A: h1' = dinv * (x @ W1), allgather
            with tc.tile_pool(name="phA", bufs=3) as sbA, \
                 tc.tile_pool(name="phA_ps", bufs=2, space="PSUM") as psA:
                for w in (range(W) if upto != "noop" else []):
                    xt = sbA.tile([P, KI, P], BF16, tag="xt")
                    nc.sync.dma_start(xt[:], xt4[:, w])
                    hp = psA.tile([P, H1], F32, tag="hp")
                    for c2 in range(KI):
                        nc.tensor.matmul(hp[:], xt[:, c2], w1sb[:, c2],
                                         start=(c2 == 0), stop=(c2 == KI - 1))
                    h1p = sbA.tile([P, H1], BF16, tag="h1p")
                    nc.scalar.activation(h1p[:], hp[:],
                                         mybir.ActivationFunctionType.Copy,
                                         scale=dinvsb[:, w:w + 1])
                    for c4 in range(4):
                        nc.sync.dma_start(ag1_in[c4][w * 32:(w + 1) * 32, :],
                                          h1p[c4 * 32:(c4 + 1) * 32, :])

            for c4 in (range(4) if upto != "noop" else []):
                if NC == 1:
                    nc.sync.dma_start(ag1_out[c4][:], ag1_in[c4][:])
                else:
                    nc.gpsimd.collective_compute(
                        "AllGather", mybir.AluOpType.bypass,
                        ins=[ag1_in[c4][:]], outs=[ag1_out[c4][:]],
                        replica_groups=rg)

            def agg_group(g, sb, sbS, accs, ag_out, F, tag,
                          skip_gather=False, skip_mm=False):
                """Gather + S-matmul segment-sum for the GRP windows of
                group g at width F, into per-window PSUM accumulators accs.
                Each acc lives in its own PSUM bank: the PE keeps GRP
                accumulation groups open at once (one per bank)."""
                wins = [g * GRP + i for i in range(GRP) if g * GRP + i < W]
                for cl in range(4):
                    nt = int(call_nt[g, cl])
                    if nt == 0:
                        continue
                    t0 = int(call_off[g, cl])
                    gbuf = sb.tile([P, NTMAX, F], BF16, tag=f"g{tag}")
                    if skip_gather:
                        nc.vector.memset(gbuf[:, :nt], 0.0)
                    else:
                        nc.gpsimd.dma_gather(
                            gbuf[:, :nt], ag_out[cl][:],
                            idxsb[:, t0 * 8:(t0 + nt) * 8],
                            nt * P, nt * P, F, single_packet=False)
                    if skip_mm:
                        continue
                    for wl, w in enumerate(wins):
                        fcl, lcl = first_last_cl(g, wl)
                        toff = int(tile_off[g, cl, wl])
                        for t in range(int(Tw[g, cl, wl])):
                            gt = toff + t
                            S = sbS.tile([P, P], BF16, tag=f"S{tag}")
                            nc.vector.tensor_tensor(
                                S[:],
                                dcolsb[:, gt:gt + 1].to_broadcast([P, P]),
                                iotasb[:], op=mybir.AluOpType.is_equal)
                            nc.tensor.matmul(
                                accs[wl][:], S[:], gbuf[:, gt - t0],
                                start=(cl == fcl and t == 0),
                                stop=(cl == lcl and t == int(Tw[g, cl, wl]) - 1))
                if skip_mm:
                    for a in accs:
                        nc.vector.memset(a[:], 0.0)

            # ---------------- phase C/D: aggregate layer1, h2' = dinv*(a1@W2)
            if upto.startswith("CD") or upto == "full":
                with tc.tile_pool(name="phC", bufs=2) as sbC, \
                     tc.tile_pool(name="phC_s", bufs=4) as sbS, \
                     tc.tile_pool(name="phC_ps", bufs=1, space="PSUM") as psC, \
                     tc.tile_pool(name="phD_ps", bufs=2, space="PSUM") as psD:
                    for g in range(NG):
                        wins = [g * GRP + i for i in range(GRP) if g * GRP + i < W]
                        accs = [psC.tile([P, H1], F32, tag=f"acc{i}",
                                         name=f"acc1_{i}")
                                for i in range(len(wins))]
                        if upto == "CD_noagg":
                            for a in accs:
                                nc.vector.memset(a[:], 0.0)
                        else:
                            agg_group(g, sbC, sbS, accs, ag1_out, H1, "1",
                                      skip_gather=(upto == "CD_nogather"),
                                      skip_mm=(upto == "CD_nomm"))
                        for wl, w in enumerate(wins):
                            z = sbC.tile([P, H1], F32, tag="z")
                            nc.vector.tensor_scalar_mul(z[:], accs[wl][:],
                                                        dinvsb[:, w:w + 1])
                            z2 = sbC.tile([P, H1], F32, tag="z2")
                            nc.vector.tensor_tensor(z2[:], z[:], b1sb[:],
                                                    op=mybir.AluOpType.add)
                            a1 = sbC.tile([P, H1], BF16, tag="a1")
                            nc.scalar.activation(a1[:], z2[:],
                                                 mybir.ActivationFunctionType.Relu)
                            if upto == "CD_nod":
                                for c4 in range(4):
                                    nc.sync.dma_start(
                                        ag2_in[c4][w * 32:(w + 1) * 32, :],
                                        a1[c4 * 32:(c4 + 1) * 32, :H2])
                                continue
                            h2p = psD.tile([P, H2], F32, tag="h2p")
                            for c2 in range(K2):
                                a1tp = psD.tile([P, P], BF16, tag="a1tp")
                                nc.tensor.transpose(a1tp[:],
                                                    a1[:, c2 * P:(c2 + 1) * P],
                                                    idsb[:])
                                a1t = sbC.tile([P, P], BF16, tag="a1t")
                                nc.vector.tensor_copy(a1t[:], a1tp[:])
                                nc.tensor.matmul(h2p[:], a1t[:], w2sb[:, c2],
                                                 start=(c2 == 0),
                                                 stop=(c2 == K2 - 1))
                            h2pp = sbC.tile([P, H2], BF16, tag="h2pp")
                            nc.scalar.activation(h2pp[:], h2p[:],
                                                 mybir.ActivationFunctionType.Copy,
                                                 scale=dinvsb[:, w:w + 1])
                            for c4 in range(4):
                                nc.sync.dma_start(
                                    ag2_in[c4][w * 32:(w + 1) * 32, :],
                                    h2pp[c4 * 32:(c4 + 1) * 32, :])

                for c4 in range(4):
                    if NC == 1:
                        nc.sync.dma_start(ag2_out[c4][:], ag2_in[c4][:])
                    else:
                        nc.gpsimd.collective_compute(
                            "AllGather", mybir.AluOpType.bypass,
                            ins=[ag2_in[c4][:]], outs=[ag2_out[c4][:]],
                            replica_groups=rg)

            # ---------------- phase E/F: aggregate layer2, logits, log_softmax
            if upto == "full":
                with tc.tile_pool(name="phE", bufs=2) as sbE, \
                     tc.tile_pool(name="phE_s", bufs=4) as sbS2, \
                     tc.tile_pool(name="phE_ps", bufs=1, space="PSUM") as psE, \
                     tc.tile_pool(name="phL_ps", bufs=2, space="PSUM") as psL:
                    for g in range(NG):
                        wins = [g * GRP + i for i in range(GRP) if g * GRP + i < W]
                        accs = [psE.tile([P, H2], F32, tag=f"acc{i}",
                                         name=f"acc2_{i}")
                                for i in range(len(wins))]
                        agg_group(g, sbE, sbS2, accs, ag2_out, H2, "2")
                        for wl, w in enumerate(wins):
                            z = sbE.tile([P, H2], F32, tag="ze")
                            nc.vector.tensor_scalar_mul(z[:], accs[wl][:],
                                                        dinvsb[:, w:w + 1])
                            z2 = sbE.tile([P, H2], F32, tag="z2e")
                            nc.vector.tensor_tensor(z2[:], z[:], b2sb[:],
                                                    op=mybir.AluOpType.add)
                            a2 = sbE.tile([P, H2], BF16, tag="a2")
                            nc.scalar.activation(a2[:], z2[:],
                                                 mybir.ActivationFunctionType.Relu)
                            a2tp = psL.tile([P, P], BF16, tag="a2tp")
                            nc.tensor.transpose(a2tp[:], a2[:], idsb[:])
                            a2t = sbE.tile([P, P], BF16, tag="a2t")
                            nc.vector.tensor_copy(a2t[:], a2tp[:])
                            lg = psL.tile([P, C], F32, tag="lg")
                            nc.tensor.matmul(lg[:], a2t[:], wlsb[:],
                                             start=True, stop=True)
                            nc.vector.tensor_tensor(lgst[:, w * C:(w + 1) * C],
                                                    lg[:], blsb[:],
                                                    op=mybir.AluOpType.add)
                            e = sbE.tile([P, C], F32, tag="e")
                            nc.scalar.activation(e[:], lgst[:, w * C:(w + 1) * C],
                                                 mybir.ActivationFunctionType.Exp,
                                                 accum_out=sst[:, w:w + 1])
                    lns = cpool.tile([P, W], F32)
                    nc.scalar.activation(lns[:], sst[:],
                                         mybir.ActivationFunctionType.Ln)
                    for w in range(W):
                        nc.vector.tensor_scalar(
                            outsb[:, w * C:(w + 1) * C], lgst[:, w * C:(w + 1) * C],
                            lns[:, w:w + 1], None, op0=mybir.AluOpType.subtract)
                    nc.sync.dma_start(outst[:], outsb[:])
            else:
                # debug variants: dummy output proving the kept phases ran
                nc.vector.memset(outsb[:], 0.0)
                if upto != "noop":
                    probe_src = ag1_out[0] if upto == "A" else ag2_out[0]
                    probe = cpool.tile([P, C], BF16)
                    nc.sync.dma_start(probe[:], probe_src[:P, :C])
                    nc.vector.tensor_copy(outsb[:, :C], probe[:])
                nc.sync.dma_start(outst[:], outsb[:])

    nc.compile()
    return nc

# ---------------------------------------------------------------- entry point
_CACHE = {}


def _get_compiled(cfg, key, meta):
    if key not in _CACHE:
        nc = build_kernel(cfg, meta)
        nc.m = get_hw_module(nc.m)
        _CACHE[key] = nc
    return _CACHE[key]


def run(cfg, inputs):
    in_maps, meta = preprocess(cfg, **inputs)
    key = (cfg.N, cfg.F_IN, meta["TT"])
    nc = _get_compiled(cfg, key, meta)
    res = bass_utils.run_bass_kernel_spmd(
        nc, in_maps, core_ids=list(range(cfg.ncores)))
    out = assemble_output(cfg, meta, res.results)
    return out, res


class _TimedRunner:
    """PJRT runner mirroring bass2jax.run_bass_via_pjrt's multi-core branch,
    but with a cached jit and device-resident inputs for repeatable timing."""

    def __init__(self, nc, n_cores):
        import jax
        import concourse.mybir as mb
        from concourse import bass2jax
        from jax.sharding import Mesh, PartitionSpec, NamedSharding
        from jax.experimental.shard_map import shard_map

        bass2jax.install_neuronx_cc_hook()
        partition_name = (nc.partition_id_tensor.name
                          if nc.partition_id_tensor else None)
        in_names, out_names, out_avals, zero_shapes = [], [], [], []
        for alloc in nc.m.functions[0].allocations:
            if not isinstance(alloc, mb.MemoryLocationSet):
                continue
            name = alloc.memorylocations[0].name
            if alloc.kind == "ExternalInput":
                if name != partition_name:
                    in_names.append(name)
            elif alloc.kind == "ExternalOutput":
                out_names.append(name)
                shape = tuple(alloc.tensor_shape)
                dtype = mb.dt.np(alloc.dtype)
                out_avals.append(jax.core.ShapedArray(shape, dtype))
                zero_shapes.append((shape, dtype))
        n_params = len(in_names)
        all_in_names = list(in_names) + list(out_names)
        if partition_name is not None:
            all_in_names.append(partition_name)
        donate = tuple(range(n_params, n_params + len(out_names)))

        def _body(*args):
            operands = list(args)
            if partition_name is not None:
                operands.append(bass2jax.partition_id_tensor())
            outs = bass2jax._bass_exec_p.bind(
                *operands,
                out_avals=tuple(out_avals),
                in_names=tuple(all_in_names),
                out_names=tuple(out_names),
                lowering_input_output_aliases=(),
                sim_require_finite=True,
                sim_require_nnan=True,
                nc=nc,
            )
            return tuple(outs)

        devices = jax.devices()[:n_cores]
        mesh = Mesh(np.asarray(devices), ("core",))
        in_specs = (PartitionSpec("core"),) * (n_params + len(out_names))
        out_specs = (PartitionSpec("core"),) * len(out_names)
        self.fn = jax.jit(
            shard_map(_body, mesh=mesh, in_specs=in_specs,
                      out_specs=out_specs, check_rep=False),
            donate_argnums=donate, keep_unused=True)
        self.jax = jax
        self.mesh = mesh
        self.sharding = NamedSharding(mesh, PartitionSpec("core"))
        self.in_names = in_names
        self.out_names = out_names
        self.zero_shapes = zero_shapes
        self.n_cores = n_cores
        self.dev_inputs = None

    def stage_inputs(self, in_maps):
        concat_in = [
            np.concatenate([np.asarray(in_maps[c][n])
                            for c in range(self.n_cores)], axis=0)
            for n in self.in_names
        ]
        self.dev_inputs = [self.jax.device_put(a, self.sharding)
                           for a in concat_in]
        for a in self.dev_inputs:
            a.block_until_ready()

    def fresh_outs(self):
        zeros = [np.zeros((self.n_cores * s[0], *s[1:]), d)
                 for s, d in self.zero_shapes]
        dz = [self.jax.device_put(z, self.sharding) for z in zeros]
        for z in dz:
            z.block_until_ready()
        return dz

    def exec_once(self):
        import time
        dz = self.fresh_outs()
        t0 = time.perf_counter()
        outs = self.fn(*self.dev_inputs, *dz)
        for o in outs:
            o.block_until_ready()
        t1 = time.perf_counter()
        return outs, t1 - t0

    def exec_chained(self, n):
        """Run n back-to-back executions (outputs fed back as donated output
        buffers), timing the whole pipeline. Amortizes host/tunnel dispatch
        overhead that a single blocking execution would include."""
        import time
        cur = self.fresh_outs()
        t0 = time.perf_counter()
        for _ in range(n):
            cur = list(self.fn(*self.dev_inputs, *cur))
        for o in cur:
            o.block_until_ready()
        t1 = time.perf_counter()
        return cur, (t1 - t0) / n

    def results(self, outs):
        res = []
        for c in range(self.n_cores):
            m = {}
            for i, n in enumerate(self.out_names):
                full = np.asarray(outs[i])
                per = full.reshape(self.n_cores, -1, *full.shape[1:])[c]
                m[n] = per
            res.append(m)
        return res


def run_timed(cfg, inputs, iters=3, chain=0):
    in_maps, meta = preprocess(cfg, **inputs)
    key = (cfg.N, cfg.F_IN, meta["TT"])
    nc = _get_compiled(cfg, key, meta)
    rkey = ("runner",) + key
    if rkey not in _CACHE:
        _CACHE[rkey] = _TimedRunner(nc, cfg.ncores)
    runner = _CACHE[rkey]
    runner.stage_inputs(in_maps)
    times = []
    outs = None
    if chain:
        runner.exec_once()  # warmup
        for _ in range(iters):
            outs, dt = runner.exec_chained(chain)
            times.append(dt)
    else:
        for _ in range(iters):
            outs, dt = runner.exec_once()
            times.append(dt)
    results = runner.results(outs)
    out = assemble_output(cfg, meta, results)
    return out, times


def kernel(x, edge_index, W1, b1, W2, b2, Wl, bl):
    out, _ = run(FULL, dict(x=x, edge_index=edge_index, W1=W1, b1=b1,
                            W2=W2, b2=b2, Wl=Wl, bl=bl))
    return out


# revision 24
# speedup vs baseline: 1.0109x; 1.0109x over previous
"""Distributed 2-layer GCN (PyG GCNConv-style) on 8 Trainium2 NeuronCores.

Strategy (hardcoded for N=100000, E=3.2M, 512->256->128->4):
  - Nodes are degree-balanced into (ncores*W) windows of 128 nodes; window b is
    owned by core (b % ncores). A node's "global position" is its row in the
    AllGathered feature table, so gathers use plain int positions.
  - Per layer: local dense matmul (bf16 on PE, fp32 PSUM), rows pre-scaled by
    dinv, results AllGathered to 4 replicated contiguous bf16 class tables in
    DRAM (class = slot//32; int16 gather indices fit because each class table
    has NPAD/4 < 32768 rows; contiguous rows gather ~13% faster than strided).
  - Aggregation: windows are processed in groups of GRP=4. Per (group, class)
    ONE large dma_gather (custom SWDGE gather, int16 indices, ~4-5k rows per
    call, single_packet=False — single_packet hangs the DMA above 1024 rows)
    pulls all the group's dst-sorted edge source rows into SBUF; per 128-edge
    tile a one-hot S matrix built on DVE (is_equal vs iota, all-bf16 for 2x
    rate) feeds a PE matmul S^T @ msgs that segment-sums into the window's
    PSUM accumulator (one PSUM bank per window: the PE keeps GRP accumulation
    groups open at once, and PSUM allows one open group per 2KB bank).
    Padding slots carry dcol=128 which never matches iota -> contribute 0.
    The gather is descriptor-bound (~9ns/row aggregate, independent of row
    bytes): per-edge HBM random reads are the kernel's floor.
  - Epilogue: z = dinv*acc + b; relu; layer 2 repeats; final logits + log
    softmax (batched Ln to avoid ACT table thrash).
"""
import math
import numpy as np

import concourse.bass as bass
import concourse.mybir as mybir
import concourse.bass_utils as bass_utils
from concourse import bacc, tile
from concourse.bass_interp import get_hw_module

P = 128
F32 = mybir.dt.float32
BF16 = mybir.dt.bfloat16
I16 = mybir.dt.int16


class Cfg:
    def __init__(self, N, F_IN, H1, H2, C, ncores=8, W=None, grp=6):
        self.N, self.F_IN, self.H1, self.H2, self.C = N, F_IN, H1, H2, C
        self.ncores = ncores
        Bc = ncores * P
        self.W = W if W is not None else math.ceil(N / Bc)
        self.NPAD = self.W * Bc
        assert self.NPAD >= N and self.NPAD % 4 == 0
        assert self.NPAD // 4 <= 32768
        self.KI = F_IN // P
        self.K2 = H1 // P
        self.B = self.W * P  # nodes per core
        self.GRP = grp
        self.NG = math.ceil(self.W / grp)


FULL = Cfg(N=100000, F_IN=512, H1=256, H2=128, C=4)


# ---------------------------------------------------------------- host side
def preprocess(cfg, x, edge_index, W1, b1, W2, b2, Wl, bl):
    N, NC, W, NPAD, B = cfg.N, cfg.ncores, cfg.W, cfg.NPAD, cfg.B
    GRP, NG = cfg.GRP, cfg.NG
    NW = NC * W

    src = np.asarray(edge_index[0], dtype=np.int64)
    dst = np.asarray(edge_index[1], dtype=np.int64)
    deg = np.bincount(dst, minlength=N).astype(np.float64) + 1.0
    dinv = (1.0 / np.sqrt(deg)).astype(np.float32)

    # node -> global position, degree-balanced across windows (snake fill)
    degall = np.zeros(NPAD, np.int64)
    degall[:N] = deg.astype(np.int64)
    order = np.argsort(-degall, kind="stable")
    i = np.arange(NPAD)
    phase = i % (2 * NW)
    binid = np.where(phase < NW, phase, 2 * NW - 1 - phase)
    by_bin = np.argsort(binid, kind="stable")
    slot = np.empty(NPAD, np.int64)
    slot[by_bin] = i % P  # within each bin, slots fill 0..127 in arrival order
    # position: core = bin % NC, window = bin // NC
    core_of_bin = binid % NC
    w_of_bin = binid // NC
    g_of_i = core_of_bin * B + w_of_bin * P + slot
    pos = np.empty(NPAD, np.int64)
    pos[order] = g_of_i

    node_at = np.empty(NPAD, np.int64)
    node_at[pos] = np.arange(NPAD)

    # edge list incl. one self edge per real node
    S_pos = np.concatenate([pos[src], pos[np.arange(N)]])
    D_pos = np.concatenate([pos[dst], pos[np.arange(N)]])
    core_e = D_pos // B
    w_e = (D_pos % B) // P
    dcol_e = (D_pos % P).astype(np.float32)
    # class tables: class = slot//32; row in class table = rank*B4 + w*32 + slot%32
    B4 = B // 4
    s_slot = S_pos % P
    cls_e = s_slot // 32
    idx16_e = ((S_pos // B) * B4 + ((S_pos % B) // P) * 32
               + (s_slot % 32)).astype(np.int16)
    g_e = w_e // GRP
    wl_e = w_e % GRP

    key = (((core_e * NG + g_e) * 4 + cls_e) * GRP + wl_e).astype(np.int64)
    ordE = np.argsort(key, kind="stable")
    counts = np.bincount(key, minlength=NC * NG * 4 * GRP).reshape(NC, NG, 4, GRP)
    Tw = np.ceil(counts / P).astype(np.int64).max(axis=0)  # [NG, 4, GRP]
    TT = int(Tw.sum())
    SLOT = TT * P

    # tile offsets in global tile order (g, cl, wl)
    tile_off = np.zeros((NG, 4, GRP), np.int64)
    acc = 0
    for g in range(NG):
        for cl in range(4):
            for wl in range(GRP):
                tile_off[g, cl, wl] = acc
                acc += int(Tw[g, cl, wl])
    call_off = tile_off[:, :, 0]                    # [NG, 4]
    call_nt = Tw.sum(axis=2)                        # [NG, 4]

    starts = np.zeros(NC * NG * 4 * GRP + 1, np.int64)
    np.cumsum(counts.reshape(-1), out=starts[1:])

    idx16 = np.zeros((NC, SLOT), np.int16)
    dcol = np.full((NC, SLOT), float(P), np.float32)
    for c in range(NC):
        for g in range(NG):
            for cl in range(4):
                for wl in range(GRP):
                    k = ((c * NG + g) * 4 + cl) * GRP + wl
                    s0, s1 = starts[k], starts[k + 1]
                    n = s1 - s0
                    if n == 0:
                        continue
                    off = int(tile_off[g, cl, wl]) * P
                    seg = ordE[s0:s1]
                    idx16[c, off:off + n] = idx16_e[seg]
                    dcol[c, off:off + n] = dcol_e[seg]

    # wrap idx16 per gather call: element i of a call at [i%16, i//16], rep x8
    idx_w = np.zeros((NC, 16, SLOT // 16), np.int16)
    for g in range(NG):
        for cl in range(4):
            off = int(call_off[g, cl]) * P
            n = int(call_nt[g, cl]) * P
            if n == 0:
                continue
            blk = idx16[:, off:off + n].reshape(NC, n // 16, 16)
            idx_w[:, :, off // 16:(off + n) // 16] = blk.transpose(0, 2, 1)
    idx_rep = np.tile(idx_w, (1, 8, 1))  # [NC, 128, SLOT//16]

    dcol_t = dcol.reshape(NC, TT, P).transpose(0, 2, 1).copy()  # [NC, 128, TT]

    # x shard, transposed chunk layout: xt4[p, w, c2, m] = x[node(w*128+m), c2*128+p]
    xpad = np.zeros((NPAD, cfg.F_IN), np.float32)
    xpad[:N] = np.asarray(x, np.float32)
    dinvpad = np.ones(NPAD, np.float32)
    dinvpad[:N] = dinv

    xt4 = np.empty((NC, P, W, cfg.KI, P), np.float32)
    dinvl = np.empty((NC, P, W), np.float32)
    for c in range(NC):
        ids = node_at[c * B:(c + 1) * B]
        xl = xpad[ids]  # [B, F_IN]
        xt4[c] = xl.reshape(W, P, cfg.KI, P).transpose(3, 0, 2, 1)
        dinvl[c] = dinvpad[ids].reshape(W, P).T

    w1d = np.asarray(W1, np.float32).reshape(cfg.KI, P, cfg.H1).transpose(1, 0, 2)
    w2d = np.asarray(W2, np.float32).reshape(cfg.K2, P, cfg.H2).transpose(1, 0, 2)
    wld = np.asarray(Wl, np.float32)  # [H2=128, C]
    b1b = np.broadcast_to(np.asarray(b1, np.float32), (P, cfg.H1)).copy()
    b2b = np.broadcast_to(np.asarray(b2, np.float32), (P, cfg.H2)).copy()
    blb = np.broadcast_to(np.asarray(bl, np.float32), (P, cfg.C)).copy()
    iota = np.broadcast_to(np.arange(P, dtype=np.float32), (P, P)).copy()
    ident = np.eye(P, dtype=np.float32)

    import ml_dtypes
    bf = lambda a: np.asarray(a, np.float32).astype(ml_dtypes.bfloat16)

    in_maps = []
    for c in range(NC):
        in_maps.append({
            "xt4": bf(xt4[c]),
            "w1d": bf(w1d), "w2d": bf(w2d), "wld": bf(wld),
            "b1b": b1b, "b2b": b2b, "blb": blb,
            "dinvl": dinvl[c],
            "idx16": idx_rep[c],
            "dcol": bf(dcol_t[c]),
            "iota": bf(iota),
            "ident": bf(ident),
        })

    meta = dict(Tw=Tw, call_off=call_off, call_nt=call_nt, tile_off=tile_off,
                pos=pos, node_at=node_at, SLOT=SLOT, TT=TT)
    return in_maps, meta


def assemble_output(cfg, meta, results):
    N, NC, W, C, B = cfg.N, cfg.ncores, cfg.W, cfg.C, cfg.B
    rows = []
    for c in range(NC):
        r = results[c]["outst"].reshape(P, W, C).transpose(1, 0, 2).reshape(B, C)
        rows.append(r)
    allrows = np.concatenate(rows, axis=0)  # [NPAD, C] in position order
    return allrows[meta["pos"][:N]].astype(np.float32)


# ---------------------------------------------------------------- device side
def build_kernel(cfg, meta, upto="full"):
    NC, W, NPAD, B = cfg.ncores, cfg.W, cfg.NPAD, cfg.B
    H1, H2, C, KI, K2 = cfg.H1, cfg.H2, cfg.C, cfg.KI, cfg.K2
    GRP, NG = cfg.GRP, cfg.NG
    Tw = np.asarray(meta["Tw"])
    call_off = np.asarray(meta["call_off"])
    call_nt = np.asarray(meta["call_nt"])
    tile_off = np.asarray(meta["tile_off"])
    TT = int(Tw.sum())
    SLOT = TT * P
    NTMAX = int(call_nt.max())

    nc = bacc.Bacc("TRN2", target_bir_lowering=False, debug=False, num_devices=NC)

    xt4 = nc.dram_tensor("xt4", [P, W, KI, P], BF16, kind="ExternalInput")
    w1d = nc.dram_tensor("w1d", [P, KI, H1], BF16, kind="ExternalInput")
    w2d = nc.dram_tensor("w2d", [P, K2, H2], BF16, kind="ExternalInput")
    wld = nc.dram_tensor("wld", [P, C], BF16, kind="ExternalInput")
    b1b = nc.dram_tensor("b1b", [P, H1], F32, kind="ExternalInput")
    b2b = nc.dram_tensor("b2b", [P, H2], F32, kind="ExternalInput")
    blb = nc.dram_tensor("blb", [P, C], F32, kind="ExternalInput")
    dinvl = nc.dram_tensor("dinvl", [P, W], F32, kind="ExternalInput")
    idx16 = nc.dram_tensor("idx16", [P, SLOT // 16], I16, kind="ExternalInput")
    dcol = nc.dram_tensor("dcol", [P, TT], BF16, kind="ExternalInput")
    iota = nc.dram_tensor("iota", [P, P], BF16, kind="ExternalInput")
    ident = nc.dram_tensor("ident", [P, P], BF16, kind="ExternalInput")
    outst = nc.dram_tensor("outst", [P, W * C], F32, kind="ExternalOutput")

    rg = [list(range(NC))]

    def first_last_cl(g, wl):
        cls = [cl for cl in range(4) if Tw[g, cl, wl] > 0]
        return cls[0], cls[-1]

    with tile.TileContext(nc) as tc:
        with tc.tile_pool(name="const", bufs=1) as cpool, \
             tc.tile_pool(name="dram", bufs=1, space="DRAM") as dram:
            ag1_in = [dram.tile([B // 4, H1], BF16, name=f"ag1i{i}")
                      for i in range(4)]
            ag1_out = [dram.tile([NPAD // 4, H1], BF16, addr_space="Shared",
                                 name=f"ag1o{i}") for i in range(4)]
            ag2_in = [dram.tile([B // 4, H2], BF16, name=f"ag2i{i}")
                      for i in range(4)]
            ag2_out = [dram.tile([NPAD // 4, H2], BF16, addr_space="Shared",
                                 name=f"ag2o{i}") for i in range(4)]

            w1sb = cpool.tile([P, KI, H1], BF16)
            nc.sync.dma_start(w1sb[:], w1d[:])
            w2sb = cpool.tile([P, K2, H2], BF16)
            nc.sync.dma_start(w2sb[:], w2d[:])
            wlsb = cpool.tile([P, C], BF16)
            nc.sync.dma_start(wlsb[:], wld[:])
            b1sb = cpool.tile([P, H1], F32)
            nc.sync.dma_start(b1sb[:], b1b[:])
            b2sb = cpool.tile([P, H2], F32)
            nc.sync.dma_start(b2sb[:], b2b[:])
            blsb = cpool.tile([P, C], F32)
            nc.sync.dma_start(blsb[:], blb[:])
            dinvsb = cpool.tile([P, W], F32)
            nc.sync.dma_start(dinvsb[:], dinvl[:])
            iotasb = cpool.tile([P, P], BF16)
            nc.sync.dma_start(iotasb[:], iota[:])
            idsb = cpool.tile([P, P], BF16)
            nc.sync.dma_start(idsb[:], ident[:])
            dcolsb = cpool.tile([P, TT], BF16)
            nc.sync.dma_start(dcolsb[:], dcol[:])
            idxsb = cpool.tile([P, SLOT // 16], I16)
            nc.sync.dma_start(idxsb[:], idx16[:])

            lgst = cpool.tile([P, W * C], F32)
            sst = cpool.tile([P, W], F32)
            outsb = cpool.tile([P, W * C], F32)

            # ---------------- phase A: h1' = dinv * (x @ W1), allgather
            with tc.tile_pool(name="phA", bufs=3) as sbA, \
                 tc.tile_pool(name="phA_ps", bufs=2, space="PSUM") as psA:
                for w in (range(W) if upto != "noop" else []):
                    xt = sbA.tile([P, KI, P], BF16, tag="xt")
                    nc.sync.dma_start(xt[:], xt4[:, w])
                    hp = psA.tile([P, H1], F32, tag="hp")
                    for c2 in range(KI):
                        nc.tensor.matmul(hp[:], xt[:, c2], w1sb[:, c2],
                                         start=(c2 == 0), stop=(c2 == KI - 1))
                    h1p = sbA.tile([P, H1], BF16, tag="h1p")
                    nc.scalar.activation(h1p[:], hp[:],
                                         mybir.ActivationFunctionType.Copy,
                                         scale=dinvsb[:, w:w + 1])
                    for c4 in range(4):
                        nc.sync.dma_start(ag1_in[c4][w * 32:(w + 1) * 32, :],
                                          h1p[c4 * 32:(c4 + 1) * 32, :])

            for c4 in (range(4) if upto != "noop" else []):
                if NC == 1:
                    nc.sync.dma_start(ag1_out[c4][:], ag1_in[c4][:])
                else:
                    nc.gpsimd.collective_compute(
                        "AllGather", mybir.AluOpType.bypass,
                        ins=[ag1_in[c4][:]], outs=[ag1_out[c4][:]],
                        replica_groups=rg)

            def agg_group(g, sb, sbS, accs, ag_out, F, tag,
                          skip_gather=False, skip_mm=False):
                """Gather + S-matmul segment-sum for the GRP windows of
                group g at width F, into per-window PSUM accumulators accs.
                Each acc lives in its own PSUM bank: the PE keeps GRP
                accumulation groups open at once (one per bank)."""
                wins = [g * GRP + i for i in range(GRP) if g * GRP + i < W]
                for cl in range(4):
                    nt = int(call_nt[g, cl])
                    if nt == 0:
                        continue
                    t0 = int(call_off[g, cl])
                    gbuf = sb.tile([P, NTMAX, F], BF16, tag=f"g{tag}")
                    if skip_gather:
                        nc.vector.memset(gbuf[:, :nt], 0.0)
                    else:
                        nc.gpsimd.dma_gather(
                            gbuf[:, :nt], ag_out[cl][:],
                            idxsb[:, t0 * 8:(t0 + nt) * 8],
                            nt * P, nt * P, F, single_packet=False)
                    if skip_mm:
                        continue
                    for wl, w in enumerate(wins):
                        fcl, lcl = first_last_cl(g, wl)
                        toff = int(tile_off[g, cl, wl])
                        for t in range(int(Tw[g, cl, wl])):
                            gt = toff + t
                            S = sbS.tile([P, P], BF16, tag=f"S{tag}")
                            nc.vector.tensor_tensor(
                                S[:],
                                dcolsb[:, gt:gt + 1].to_broadcast([P, P]),
                                iotasb[:], op=mybir.AluOpType.is_equal)
                            nc.tensor.matmul(
                                accs[wl][:], S[:], gbuf[:, gt - t0],
                                start=(cl == fcl and t == 0),
                                stop=(cl == lcl and t == int(Tw[g, cl, wl]) - 1))
                if skip_mm:
                    for a in accs:
                        nc.vector.memset(a[:], 0.0)

            # ---------------- phase C/D: aggregate layer1, h2' = dinv*(a1@W2)
            if upto.startswith("CD") or upto == "full":
                with tc.tile_pool(name="phC", bufs=2) as sbC, \
                     tc.tile_pool(name="phC_s", bufs=4) as sbS, \
                     tc.tile_pool(name="phC_ps", bufs=1, space="PSUM") as psC, \
                     tc.tile_pool(name="phD_ps", bufs=1, space="PSUM") as psD:
                    for g in range(NG):
                        wins = [g * GRP + i for i in range(GRP) if g * GRP + i < W]
                        accs = [psC.tile([P, H1], F32, tag=f"acc{i}",
                                         name=f"acc1_{i}")
                                for i in range(len(wins))]
                        if upto == "CD_noagg":
                            for a in accs:
                                nc.vector.memset(a[:], 0.0)
                        else:
                            agg_group(g, sbC, sbS, accs, ag1_out, H1, "1",
                                      skip_gather=(upto == "CD_nogather"),
                                      skip_mm=(upto == "CD_nomm"))
                        for wl, w in enumerate(wins):
                            z = sbC.tile([P, H1], F32, tag="z")
                            nc.vector.tensor_scalar_mul(z[:], accs[wl][:],
                                                        dinvsb[:, w:w + 1])
                            z2 = sbC.tile([P, H1], F32, tag="z2")
                            nc.vector.tensor_tensor(z2[:], z[:], b1sb[:],
                                                    op=mybir.AluOpType.add)
                            a1 = sbC.tile([P, H1], BF16, tag="a1")
                            nc.scalar.activation(a1[:], z2[:],
                                                 mybir.ActivationFunctionType.Relu)
                            if upto == "CD_nod":
                                for c4 in range(4):
                                    nc.sync.dma_start(
                                        ag2_in[c4][w * 32:(w + 1) * 32, :],
                                        a1[c4 * 32:(c4 + 1) * 32, :H2])
                                continue
                            h2p = psD.tile([P, H2], F32, tag="h2p")
                            for c2 in range(K2):
                                a1tp = psD.tile([P, P], BF16, tag="a1tp")
                                nc.tensor.transpose(a1tp[:],
                                                    a1[:, c2 * P:(c2 + 1) * P],
                                                    idsb[:])
                                a1t = sbC.tile([P, P], BF16, tag="a1t")
                                nc.vector.tensor_copy(a1t[:], a1tp[:])
                                nc.tensor.matmul(h2p[:], a1t[:], w2sb[:, c2],
                                                 start=(c2 == 0),
                                                 stop=(c2 == K2 - 1))
                            h2pp = sbC.tile([P, H2], BF16, tag="h2pp")
                            nc.scalar.activation(h2pp[:], h2p[:],
                                                 mybir.ActivationFunctionType.Copy,
                                                 scale=dinvsb[:, w:w + 1])
                            for c4 in range(4):
                                nc.sync.dma_start(
                                    ag2_in[c4][w * 32:(w + 1) * 32, :],
                                    h2pp[c4 * 32:(c4 + 1) * 32, :])

                for c4 in range(4):
                    if NC == 1:
                        nc.sync.dma_start(ag2_out[c4][:], ag2_in[c4][:])
                    else:
                        nc.gpsimd.collective_compute(
                            "AllGather", mybir.AluOpType.bypass,
                            ins=[ag2_in[c4][:]], outs=[ag2_out[c4][:]],
                            replica_groups=rg)

            # ---------------- phase E/F: aggregate layer2, logits, log_softmax
            if upto == "full":
                with tc.tile_pool(name="phE", bufs=2) as sbE, \
                     tc.tile_pool(name="phE_s", bufs=4) as sbS2, \
                     tc.tile_pool(name="phE_ps", bufs=1, space="PSUM") as psE, \
                     tc.tile_pool(name="phL_ps", bufs=1, space="PSUM") as psL:
                    for g in range(NG):
                        wins = [g * GRP + i for i in range(GRP) if g * GRP + i < W]
                        accs = [psE.tile([P, H2], F32, tag=f"acc{i}",
                                         name=f"acc2_{i}")
                                for i in range(len(wins))]
                        agg_group(g, sbE, sbS2, accs, ag2_out, H2, "2")
                        for wl, w in enumerate(wins):
                            z = sbE.tile([P, H2], F32, tag="ze")
                            nc.vector.tensor_scalar_mul(z[:], accs[wl][:],
                                                        dinvsb[:, w:w + 1])
                            z2 = sbE.tile([P, H2], F32, tag="z2e")
                            nc.vector.tensor_tensor(z2[:], z[:], b2sb[:],
                                                    op=mybir.AluOpType.add)
                            a2 = sbE.tile([P, H2], BF16, tag="a2")
                            nc.scalar.activation(a2[:], z2[:],
                                                 mybir.ActivationFunctionType.Relu)
                            a2tp = psL.tile([P, P], BF16, tag="a2tp")
                            nc.tensor.transpose(a2tp[:], a2[:], idsb[:])
                            a2t = sbE.tile([P, P], BF16, tag="a2t")
                            nc.vector.tensor_copy(a2t[:], a2tp[:])
                            lg = psL.tile([P, C], F32, tag="lg")
                            nc.tensor.matmul(lg[:], a2t[:], wlsb[:],
                                             start=True, stop=True)
                            nc.vector.tensor_tensor(lgst[:, w * C:(w + 1) * C],
                                                    lg[:], blsb[:],
                                                    op=mybir.AluOpType.add)
                            e = sbE.tile([P, C], F32, tag="e")
                            nc.scalar.activation(e[:], lgst[:, w * C:(w + 1) * C],
                                                 mybir.ActivationFunctionType.Exp,
                                                 accum_out=sst[:, w:w + 1])
                    lns = cpool.tile([P, W], F32)
                    nc.scalar.activation(lns[:], sst[:],
                                         mybir.ActivationFunctionType.Ln)
                    for w in range(W):
                        nc.vector.tensor_scalar(
                            outsb[:, w * C:(w + 1) * C], lgst[:, w * C:(w + 1) * C],
                            lns[:, w:w + 1], None, op0=mybir.AluOpType.subtract)
                    nc.sync.dma_start(outst[:], outsb[:])
            else:
                # debug variants: dummy output proving the kept phases ran
                nc.vector.memset(outsb[:], 0.0)
                if upto != "noop":
                    probe_src = ag1_out[0] if upto == "A" else ag2_out[0]
                    probe = cpool.tile([P, C], BF16)
                    nc.sync.dma_start(probe[:], probe_src[:P, :C])
                    nc.vector.tensor_copy(outsb[:, :C], probe[:])
                nc.sync.dma_start(outst[:], outsb[:])

    nc.compile()
    return nc

# ---------------------------------------------------------------- entry point
_CACHE = {}


def _get_compiled(cfg, key, meta):
    if key not in _CACHE:
        nc = build_kernel(cfg, meta)
        nc.m = get_hw_module(nc.m)
        _CACHE[key] = nc
    return _CACHE[key]


def run(cfg, inputs):
    in_maps, meta = preprocess(cfg, **inputs)
    key = (cfg.N, cfg.F_IN, meta["TT"])
    nc = _get_compiled(cfg, key, meta)
    res = bass_utils.run_bass_kernel_spmd(
        nc, in_maps, core_ids=list(range(cfg.ncores)))
    out = assemble_output(cfg, meta, res.results)
    return out, res


class _TimedRunner:
    """PJRT runner mirroring bass2jax.run_bass_via_pjrt's multi-core branch,
    but with a cached jit and device-resident inputs for repeatable timing."""

    def __init__(self, nc, n_cores):
        import jax
        import concourse.mybir as mb
        from concourse import bass2jax
        from jax.sharding import Mesh, PartitionSpec, NamedSharding
        from jax.experimental.shard_map import shard_map

        bass2jax.install_neuronx_cc_hook()
        partition_name = (nc.partition_id_tensor.name
                          if nc.partition_id_tensor else None)
        in_names, out_names, out_avals, zero_shapes = [], [], [], []
        for alloc in nc.m.functions[0].allocations:
            if not isinstance(alloc, mb.MemoryLocationSet):
                continue
            name = alloc.memorylocations[0].name
            if alloc.kind == "ExternalInput":
                if name != partition_name:
                    in_names.append(name)
            elif alloc.kind == "ExternalOutput":
                out_names.append(name)
                shape = tuple(alloc.tensor_shape)
                dtype = mb.dt.np(alloc.dtype)
                out_avals.append(jax.core.ShapedArray(shape, dtype))
                zero_shapes.append((shape, dtype))
        n_params = len(in_names)
        all_in_names = list(in_names) + list(out_names)
        if partition_name is not None:
            all_in_names.append(partition_name)
        donate = tuple(range(n_params, n_params + len(out_names)))

        def _body(*args):
            operands = list(args)
            if partition_name is not None:
                operands.append(bass2jax.partition_id_tensor())
            outs = bass2jax._bass_exec_p.bind(
                *operands,
                out_avals=tuple(out_avals),
                in_names=tuple(all_in_names),
                out_names=tuple(out_names),
                lowering_input_output_aliases=(),
                sim_require_finite=True,
                sim_require_nnan=True,
                nc=nc,
            )
            return tuple(outs)

        devices = jax.devices()[:n_cores]
        mesh = Mesh(np.asarray(devices), ("core",))
        in_specs = (PartitionSpec("core"),) * (n_params + len(out_names))
        out_specs = (PartitionSpec("core"),) * len(out_names)
        # No donation: output buffers are not aliased, so back-to-back
        # executions have no host-side data dependency and submissions can
        # pipeline ahead of device execution (the device serializes them).
        del donate
        self.fn = jax.jit(
            shard_map(_body, mesh=mesh, in_specs=in_specs,
                      out_specs=out_specs, check_rep=False),
            keep_unused=True)
        self.jax = jax
        self.mesh = mesh
        self.sharding = NamedSharding(mesh, PartitionSpec("core"))
        self.in_names = in_names
        self.out_names = out_names
        self.zero_shapes = zero_shapes
        self.n_cores = n_cores
        self.dev_inputs = None

    def stage_inputs(self, in_maps):
        concat_in = [
            np.concatenate([np.asarray(in_maps[c][n])
                            for c in range(self.n_cores)], axis=0)
            for n in self.in_names
        ]
        self.dev_inputs = [self.jax.device_put(a, self.sharding)
                           for a in concat_in]
        for a in self.dev_inputs:
            a.block_until_ready()

    def fresh_outs(self):
        zeros = [np.zeros((self.n_cores * s[0], *s[1:]), d)
                 for s, d in self.zero_shapes]
        dz = [self.jax.device_put(z, self.sharding) for z in zeros]
        for z in dz:
            z.block_until_ready()
        return dz

    def exec_once(self):
        import time
        dz = self.fresh_outs()
        t0 = time.perf_counter()
        outs = self.fn(*self.dev_inputs, *dz)
        for o in outs:
            o.block_until_ready()
        t1 = time.perf_counter()
        return outs, t1 - t0

    def exec_chained(self, n):
        """Run n back-to-back executions, timing the whole pipeline and
        dividing by n. Executions are independent (same device-resident
        inputs, no donation), so host dispatch can pipeline ahead while the
        device executes them serially — per-iter converges to the actual
        per-execution hardware time, amortizing the host/tunnel dispatch
        overhead that a single blocking execution would include."""
        import time
        dz = self.fresh_outs()
        cur = None
        t0 = time.perf_counter()
        for _ in range(n):
            cur = list(self.fn(*self.dev_inputs, *dz))
        for o in cur:
            o.block_until_ready()
        t1 = time.perf_counter()
        return cur, (t1 - t0) / n

    def results(self, outs):
        res = []
        for c in range(self.n_cores):
            m = {}
            for i, n in enumerate(self.out_names):
                full = np.asarray(outs[i])
                per = full.reshape(self.n_cores, -1, *full.shape[1:])[c]
                m[n] = per
            res.append(m)
        return res


def run_timed(cfg, inputs, iters=3, chain=0):
    in_maps, meta = preprocess(cfg, **inputs)
    key = (cfg.N, cfg.F_IN, meta["TT"])
    nc = _get_compiled(cfg, key, meta)
    rkey = ("runner",) + key
    if rkey not in _CACHE:
        _CACHE[rkey] = _TimedRunner(nc, cfg.ncores)
    runner = _CACHE[rkey]
    runner.stage_inputs(in_maps)
    times = []
    outs = None
    if chain:
        runner.exec_once()  # warmup
        for _ in range(iters):
            outs, dt = runner.exec_chained(chain)
            times.append(dt)
    else:
        for _ in range(iters):
            outs, dt = runner.exec_once()
            times.append(dt)
    results = runner.results(outs)
    out = assemble_output(cfg, meta, results)
    return out, times


def kernel(x, edge_index, W1, b1, W2, b2, Wl, bl):
    out, _ = run(FULL, dict(x=x, edge_index=edge_index, W1=W1, b1=b1,
                            W2=W2, b2=b2, Wl=Wl, bl=bl))
    return out


# revision 28
# speedup vs baseline: 1.3505x; 1.3359x over previous
"""Distributed 2-layer GCN (PyG GCNConv-style) on 8 Trainium2 NeuronCores.

Strategy (hardcoded for N=100000, E=3.2M, 512->256->128->4):
  - Nodes are degree-balanced into (ncores*W) windows of 128 nodes; window b is
    owned by core (b % ncores). A node's "global position" is its row in the
    AllGathered feature table, so gathers use plain int positions.
  - Per layer: local dense matmul (bf16 on PE, fp32 PSUM), rows pre-scaled by
    dinv, results AllGathered to 4 replicated contiguous bf16 class tables in
    DRAM (class = slot//32; int16 gather indices fit because each class table
    has NPAD/4 < 32768 rows; contiguous rows gather ~13% faster than strided).
  - Aggregation: windows are processed in groups of GRP=4. Per (group, class)
    ONE large dma_gather (custom SWDGE gather, int16 indices, ~4-5k rows per
    call, single_packet=False — single_packet hangs the DMA above 1024 rows)
    pulls all the group's dst-sorted edge source rows into SBUF; per 128-edge
    tile a one-hot S matrix built on DVE (is_equal vs iota, all-bf16 for 2x
    rate) feeds a PE matmul S^T @ msgs that segment-sums into the window's
    PSUM accumulator (one PSUM bank per window: the PE keeps GRP accumulation
    groups open at once, and PSUM allows one open group per 2KB bank).
    Padding slots carry dcol=128 which never matches iota -> contribute 0.
    The gather is descriptor-bound (~9ns/row aggregate, independent of row
    bytes): per-edge HBM random reads are the kernel's floor.
  - Epilogue: z = dinv*acc + b; relu; layer 2 repeats; final logits + log
    softmax (batched Ln to avoid ACT table thrash).
"""
import math
import numpy as np

import concourse.bass as bass
import concourse.mybir as mybir
import concourse.bass_utils as bass_utils
from concourse import bacc, tile
from concourse.bass_interp import get_hw_module

P = 128
F32 = mybir.dt.float32
BF16 = mybir.dt.bfloat16
I16 = mybir.dt.int16


class Cfg:
    def __init__(self, N, F_IN, H1, H2, C, ncores=8, W=None, grp=6):
        self.N, self.F_IN, self.H1, self.H2, self.C = N, F_IN, H1, H2, C
        self.ncores = ncores
        Bc = ncores * P
        self.W = W if W is not None else math.ceil(N / Bc)
        self.NPAD = self.W * Bc
        assert self.NPAD >= N and self.NPAD % 4 == 0
        assert self.NPAD // 4 <= 32768
        self.KI = F_IN // P
        self.K2 = H1 // P
        self.B = self.W * P  # nodes per core
        self.GRP = grp
        self.NG = math.ceil(self.W / grp)


FULL = Cfg(N=100000, F_IN=512, H1=256, H2=128, C=4)


# ---------------------------------------------------------------- host side
def preprocess(cfg, x, edge_index, W1, b1, W2, b2, Wl, bl):
    N, NC, W, NPAD, B = cfg.N, cfg.ncores, cfg.W, cfg.NPAD, cfg.B
    GRP, NG = cfg.GRP, cfg.NG
    NW = NC * W

    src = np.asarray(edge_index[0], dtype=np.int64)
    dst = np.asarray(edge_index[1], dtype=np.int64)
    deg = np.bincount(dst, minlength=N).astype(np.float64) + 1.0
    dinv = (1.0 / np.sqrt(deg)).astype(np.float32)

    # node -> global position, degree-balanced across windows (snake fill)
    degall = np.zeros(NPAD, np.int64)
    degall[:N] = deg.astype(np.int64)
    order = np.argsort(-degall, kind="stable")
    i = np.arange(NPAD)
    phase = i % (2 * NW)
    binid = np.where(phase < NW, phase, 2 * NW - 1 - phase)
    by_bin = np.argsort(binid, kind="stable")
    slot = np.empty(NPAD, np.int64)
    slot[by_bin] = i % P  # within each bin, slots fill 0..127 in arrival order
    # position: core = bin % NC, window = bin // NC
    core_of_bin = binid % NC
    w_of_bin = binid // NC
    g_of_i = core_of_bin * B + w_of_bin * P + slot
    pos = np.empty(NPAD, np.int64)
    pos[order] = g_of_i

    node_at = np.empty(NPAD, np.int64)
    node_at[pos] = np.arange(NPAD)

    # edge list incl. one self edge per real node
    S_pos = np.concatenate([pos[src], pos[np.arange(N)]])
    D_pos = np.concatenate([pos[dst], pos[np.arange(N)]])
    core_e = D_pos // B
    w_e = (D_pos % B) // P
    dcol_e = (D_pos % P).astype(np.float32)
    # class tables: class = slot//32; row in class table = rank*B4 + w*32 + slot%32
    B4 = B // 4
    s_slot = S_pos % P
    cls_e = s_slot // 32
    idx16_e = ((S_pos // B) * B4 + ((S_pos % B) // P) * 32
               + (s_slot % 32)).astype(np.int16)
    g_e = w_e // GRP
    wl_e = w_e % GRP

    key = (((core_e * NG + g_e) * 4 + cls_e) * GRP + wl_e).astype(np.int64)
    ordE = np.argsort(key, kind="stable")
    counts = np.bincount(key, minlength=NC * NG * 4 * GRP).reshape(NC, NG, 4, GRP)
    Tw = np.ceil(counts / P).astype(np.int64).max(axis=0)  # [NG, 4, GRP]
    TT = int(Tw.sum())
    SLOT = TT * P

    # tile offsets in global tile order (g, cl, wl)
    tile_off = np.zeros((NG, 4, GRP), np.int64)
    acc = 0
    for g in range(NG):
        for cl in range(4):
            for wl in range(GRP):
                tile_off[g, cl, wl] = acc
                acc += int(Tw[g, cl, wl])
    call_off = tile_off[:, :, 0]                    # [NG, 4]
    call_nt = Tw.sum(axis=2)                        # [NG, 4]

    starts = np.zeros(NC * NG * 4 * GRP + 1, np.int64)
    np.cumsum(counts.reshape(-1), out=starts[1:])

    idx16 = np.zeros((NC, SLOT), np.int16)
    dcol = np.full((NC, SLOT), float(P), np.float32)
    for c in range(NC):
        for g in range(NG):
            for cl in range(4):
                for wl in range(GRP):
                    k = ((c * NG + g) * 4 + cl) * GRP + wl
                    s0, s1 = starts[k], starts[k + 1]
                    n = s1 - s0
                    if n == 0:
                        continue
                    off = int(tile_off[g, cl, wl]) * P
                    seg = ordE[s0:s1]
                    idx16[c, off:off + n] = idx16_e[seg]
                    dcol[c, off:off + n] = dcol_e[seg]

    # wrap idx16 per gather call: element i of a call at [i%16, i//16], rep x8
    idx_w = np.zeros((NC, 16, SLOT // 16), np.int16)
    for g in range(NG):
        for cl in range(4):
            off = int(call_off[g, cl]) * P
            n = int(call_nt[g, cl]) * P
            if n == 0:
                continue
            blk = idx16[:, off:off + n].reshape(NC, n // 16, 16)
            idx_w[:, :, off // 16:(off + n) // 16] = blk.transpose(0, 2, 1)
    idx_rep = np.tile(idx_w, (1, 8, 1))  # [NC, 128, SLOT//16]

    dcol_t = dcol.reshape(NC, TT, P).transpose(0, 2, 1).copy()  # [NC, 128, TT]

    # x shard, transposed chunk layout: xt4[p, w, c2, m] = x[node(w*128+m), c2*128+p]
    xpad = np.zeros((NPAD, cfg.F_IN), np.float32)
    xpad[:N] = np.asarray(x, np.float32)
    dinvpad = np.ones(NPAD, np.float32)
    dinvpad[:N] = dinv

    xt4 = np.empty((NC, P, W, cfg.KI, P), np.float32)
    dinvl = np.empty((NC, P, W), np.float32)
    for c in range(NC):
        ids = node_at[c * B:(c + 1) * B]
        xl = xpad[ids]  # [B, F_IN]
        xt4[c] = xl.reshape(W, P, cfg.KI, P).transpose(3, 0, 2, 1)
        dinvl[c] = dinvpad[ids].reshape(W, P).T

    w1d = np.asarray(W1, np.float32).reshape(cfg.KI, P, cfg.H1).transpose(1, 0, 2)
    w2d = np.asarray(W2, np.float32).reshape(cfg.K2, P, cfg.H2).transpose(1, 0, 2)
    wld = np.asarray(Wl, np.float32)  # [H2=128, C]
    b1b = np.broadcast_to(np.asarray(b1, np.float32), (P, cfg.H1)).copy()
    b2b = np.broadcast_to(np.asarray(b2, np.float32), (P, cfg.H2)).copy()
    blb = np.broadcast_to(np.asarray(bl, np.float32), (P, cfg.C)).copy()
    iota = np.broadcast_to(np.arange(P, dtype=np.float32), (P, P)).copy()
    ident = np.eye(P, dtype=np.float32)

    import ml_dtypes
    bf = lambda a: np.asarray(a, np.float32).astype(ml_dtypes.bfloat16)

    in_maps = []
    for c in range(NC):
        in_maps.append({
            "xt4": bf(xt4[c]),
            "w1d": bf(w1d), "w2d": bf(w2d), "wld": bf(wld),
            "b1b": b1b, "b2b": b2b, "blb": blb,
            "dinvl": dinvl[c],
            "idx16": idx_rep[c],
            "dcol": bf(dcol_t[c]),
            "iota": bf(iota),
            "ident": bf(ident),
        })

    meta = dict(Tw=Tw, call_off=call_off, call_nt=call_nt, tile_off=tile_off,
                pos=pos, node_at=node_at, SLOT=SLOT, TT=TT)
    return in_maps, meta


def assemble_output(cfg, meta, results):
    N, NC, W, C, B = cfg.N, cfg.ncores, cfg.W, cfg.C, cfg.B
    rows = []
    for c in range(NC):
        r = results[c]["outst"].reshape(P, W, C).transpose(1, 0, 2).reshape(B, C)
        rows.append(r)
    allrows = np.concatenate(rows, axis=0)  # [NPAD, C] in position order
    return allrows[meta["pos"][:N]].astype(np.float32)


# ---------------------------------------------------------------- device side
def build_kernel(cfg, meta, upto="full"):
    NC, W, NPAD, B = cfg.ncores, cfg.W, cfg.NPAD, cfg.B
    H1, H2, C, KI, K2 = cfg.H1, cfg.H2, cfg.C, cfg.KI, cfg.K2
    GRP, NG = cfg.GRP, cfg.NG
    Tw = np.asarray(meta["Tw"])
    call_off = np.asarray(meta["call_off"])
    call_nt = np.asarray(meta["call_nt"])
    tile_off = np.asarray(meta["tile_off"])
    TT = int(Tw.sum())
    SLOT = TT * P
    NTMAX = int(call_nt.max())

    nc = bacc.Bacc("TRN2", target_bir_lowering=False, debug=False, num_devices=NC)

    xt4 = nc.dram_tensor("xt4", [P, W, KI, P], BF16, kind="ExternalInput")
    w1d = nc.dram_tensor("w1d", [P, KI, H1], BF16, kind="ExternalInput")
    w2d = nc.dram_tensor("w2d", [P, K2, H2], BF16, kind="ExternalInput")
    wld = nc.dram_tensor("wld", [P, C], BF16, kind="ExternalInput")
    b1b = nc.dram_tensor("b1b", [P, H1], F32, kind="ExternalInput")
    b2b = nc.dram_tensor("b2b", [P, H2], F32, kind="ExternalInput")
    blb = nc.dram_tensor("blb", [P, C], F32, kind="ExternalInput")
    dinvl = nc.dram_tensor("dinvl", [P, W], F32, kind="ExternalInput")
    idx16 = nc.dram_tensor("idx16", [P, SLOT // 16], I16, kind="ExternalInput")
    dcol = nc.dram_tensor("dcol", [P, TT], BF16, kind="ExternalInput")
    iota = nc.dram_tensor("iota", [P, P], BF16, kind="ExternalInput")
    ident = nc.dram_tensor("ident", [P, P], BF16, kind="ExternalInput")
    outst = nc.dram_tensor("outst", [P, W * C], F32, kind="ExternalOutput")

    rg = [list(range(NC))]

    def first_last_cl(g, wl):
        cls = [cl for cl in range(4) if Tw[g, cl, wl] > 0]
        return cls[0], cls[-1]

    with tile.TileContext(nc) as tc:
        with tc.tile_pool(name="const", bufs=1) as cpool, \
             tc.tile_pool(name="dram", bufs=1, space="DRAM") as dram:
            ag1_in = [dram.tile([B // 4, H1], BF16, name=f"ag1i{i}")
                      for i in range(4)]
            ag1_out = [dram.tile([NPAD // 4, H1], BF16, addr_space="Shared",
                                 name=f"ag1o{i}") for i in range(4)]
            ag2_in = [dram.tile([B // 4, H2], BF16, name=f"ag2i{i}")
                      for i in range(4)]
            ag2_out = [dram.tile([NPAD // 4, H2], BF16, addr_space="Shared",
                                 name=f"ag2o{i}") for i in range(4)]

            w1sb = cpool.tile([P, KI, H1], BF16)
            nc.sync.dma_start(w1sb[:], w1d[:])
            w2sb = cpool.tile([P, K2, H2], BF16)
            nc.sync.dma_start(w2sb[:], w2d[:])
            wlsb = cpool.tile([P, C], BF16)
            nc.sync.dma_start(wlsb[:], wld[:])
            b1sb = cpool.tile([P, H1], F32)
            nc.sync.dma_start(b1sb[:], b1b[:])
            b2sb = cpool.tile([P, H2], F32)
            nc.sync.dma_start(b2sb[:], b2b[:])
            blsb = cpool.tile([P, C], F32)
            nc.sync.dma_start(blsb[:], blb[:])
            dinvsb = cpool.tile([P, W], F32)
            nc.sync.dma_start(dinvsb[:], dinvl[:])
            iotasb = cpool.tile([P, P], BF16)
            nc.sync.dma_start(iotasb[:], iota[:])
            idsb = cpool.tile([P, P], BF16)
            nc.sync.dma_start(idsb[:], ident[:])
            dcolsb = cpool.tile([P, TT], BF16)
            nc.sync.dma_start(dcolsb[:], dcol[:])
            idxsb = cpool.tile([P, SLOT // 16], I16)
            nc.sync.dma_start(idxsb[:], idx16[:])

            lgst = cpool.tile([P, W * C], F32)
            sst = cpool.tile([P, W], F32)
            outsb = cpool.tile([P, W * C], F32)

            # ---------------- phase A: h1' = dinv * (x @ W1), allgather
            with tc.tile_pool(name="phA", bufs=3) as sbA, \
                 tc.tile_pool(name="phA_ps", bufs=2, space="PSUM") as psA:
                for w in (range(W) if upto != "noop" else []):
                    xt = sbA.tile([P, KI, P], BF16, tag="xt")
                    nc.sync.dma_start(xt[:], xt4[:, w])
                    hp = psA.tile([P, H1], F32, tag="hp")
                    for c2 in range(KI):
                        nc.tensor.matmul(hp[:], xt[:, c2], w1sb[:, c2],
                                         start=(c2 == 0), stop=(c2 == KI - 1))
                    h1p = sbA.tile([P, H1], BF16, tag="h1p")
                    nc.scalar.activation(h1p[:], hp[:],
                                         mybir.ActivationFunctionType.Copy,
                                         scale=dinvsb[:, w:w + 1])
                    for c4 in range(4):
                        nc.sync.dma_start(ag1_in[c4][w * 32:(w + 1) * 32, :],
                                          h1p[c4 * 32:(c4 + 1) * 32, :])

            for c4 in (range(4) if upto != "noop" else []):
                if NC == 1:
                    nc.sync.dma_start(ag1_out[c4][:], ag1_in[c4][:])
                else:
                    nc.gpsimd.collective_compute(
                        "AllGather", mybir.AluOpType.bypass,
                        ins=[ag1_in[c4][:]], outs=[ag1_out[c4][:]],
                        replica_groups=rg)

            def agg_group(g, sb, sbS, accs, ag_out, F, tag,
                          skip_gather=False, skip_mm=False):
                """Gather + S-matmul segment-sum for the GRP windows of
                group g at width F, into per-window PSUM accumulators accs.
                Each acc lives in its own PSUM bank: the PE keeps GRP
                accumulation groups open at once (one per bank)."""
                wins = [g * GRP + i for i in range(GRP) if g * GRP + i < W]
                for cl in range(4):
                    nt = int(call_nt[g, cl])
                    if nt == 0:
                        continue
                    t0 = int(call_off[g, cl])
                    gbuf = sb.tile([P, NTMAX, F], BF16, tag=f"g{tag}")
                    if skip_gather:
                        nc.vector.memset(gbuf[:, :nt], 0.0)
                    else:
                        nc.gpsimd.dma_gather(
                            gbuf[:, :nt], ag_out[cl][:],
                            idxsb[:, t0 * 8:(t0 + nt) * 8],
                            nt * P, nt * P, F, single_packet=False)
                    if skip_mm:
                        continue
                    for wl, w in enumerate(wins):
                        fcl, lcl = first_last_cl(g, wl)
                        toff = int(tile_off[g, cl, wl])
                        for t in range(int(Tw[g, cl, wl])):
                            gt = toff + t
                            S = sbS.tile([P, P], BF16, tag=f"S{tag}")
                            nc.vector.tensor_tensor(
                                S[:],
                                dcolsb[:, gt:gt + 1].to_broadcast([P, P]),
                                iotasb[:], op=mybir.AluOpType.is_equal)
                            nc.tensor.matmul(
                                accs[wl][:], S[:], gbuf[:, gt - t0],
                                start=(cl == fcl and t == 0),
                                stop=(cl == lcl and t == int(Tw[g, cl, wl]) - 1))
                if skip_mm:
                    for a in accs:
                        nc.vector.memset(a[:], 0.0)

            # ---------------- phase C/D: aggregate layer1, h2' = dinv*(a1@W2)
            if upto.startswith("CD") or upto == "full":
                with tc.tile_pool(name="phC", bufs=2) as sbC, \
                     tc.tile_pool(name="phC_s", bufs=4) as sbS, \
                     tc.tile_pool(name="phC_ps", bufs=1, space="PSUM") as psC, \
                     tc.tile_pool(name="phD_ps", bufs=1, space="PSUM") as psD:
                    for g in range(NG):
                        wins = [g * GRP + i for i in range(GRP) if g * GRP + i < W]
                        accs = [psC.tile([P, H1], F32, tag=f"acc{i}",
                                         name=f"acc1_{i}")
                                for i in range(len(wins))]
                        if upto == "CD_noagg":
                            for a in accs:
                                nc.vector.memset(a[:], 0.0)
                        else:
                            agg_group(g, sbC, sbS, accs, ag1_out, H1, "1",
                                      skip_gather=(upto == "CD_nogather"),
                                      skip_mm=(upto == "CD_nomm"))
                        for wl, w in enumerate(wins):
                            z = sbC.tile([P, H1], F32, tag="z")
                            nc.vector.tensor_scalar_mul(z[:], accs[wl][:],
                                                        dinvsb[:, w:w + 1])
                            z2 = sbC.tile([P, H1], F32, tag="z2")
                            nc.vector.tensor_tensor(z2[:], z[:], b1sb[:],
                                                    op=mybir.AluOpType.add)
                            a1 = sbC.tile([P, H1], BF16, tag="a1")
                            nc.scalar.activation(a1[:], z2[:],
                                                 mybir.ActivationFunctionType.Relu)
                            if upto == "CD_nod":
                                for c4 in range(4):
                                    nc.sync.dma_start(
                                        ag2_in[c4][w * 32:(w + 1) * 32, :],
                                        a1[c4 * 32:(c4 + 1) * 32, :H2])
                                continue
                            h2p = psD.tile([P, H2], F32, tag="h2p")
                            for c2 in range(K2):
                                a1tp = psD.tile([P, P], BF16, tag="a1tp")
                                nc.tensor.transpose(a1tp[:],
                                                    a1[:, c2 * P:(c2 + 1) * P],
                                                    idsb[:])
                                a1t = sbC.tile([P, P], BF16, tag="a1t")
                                nc.vector.tensor_copy(a1t[:], a1tp[:])
                                nc.tensor.matmul(h2p[:], a1t[:], w2sb[:, c2],
                                                 start=(c2 == 0),
                                                 stop=(c2 == K2 - 1))
                            h2pp = sbC.tile([P, H2], BF16, tag="h2pp")
                            nc.scalar.activation(h2pp[:], h2p[:],
                                                 mybir.ActivationFunctionType.Copy,
                                                 scale=dinvsb[:, w:w + 1])
                            for c4 in range(4):
                                nc.sync.dma_start(
                                    ag2_in[c4][w * 32:(w + 1) * 32, :],
                                    h2pp[c4 * 32:(c4 + 1) * 32, :])

                for c4 in range(4):
                    if NC == 1:
                        nc.sync.dma_start(ag2_out[c4][:], ag2_in[c4][:])
                    else:
                        nc.gpsimd.collective_compute(
                            "AllGather", mybir.AluOpType.bypass,
                            ins=[ag2_in[c4][:]], outs=[ag2_out[c4][:]],
                            replica_groups=rg)

            # ---------------- phase E/F: aggregate layer2, logits, log_softmax
            if upto == "full":
                with tc.tile_pool(name="phE", bufs=2) as sbE, \
                     tc.tile_pool(name="phE_s", bufs=4) as sbS2, \
                     tc.tile_pool(name="phE_ps", bufs=1, space="PSUM") as psE, \
                     tc.tile_pool(name="phL_ps", bufs=1, space="PSUM") as psL:
                    for g in range(NG):
                        wins = [g * GRP + i for i in range(GRP) if g * GRP + i < W]
                        accs = [psE.tile([P, H2], F32, tag=f"acc{i}",
                                         name=f"acc2_{i}")
                                for i in range(len(wins))]
                        agg_group(g, sbE, sbS2, accs, ag2_out, H2, "2")
                        for wl, w in enumerate(wins):
                            z = sbE.tile([P, H2], F32, tag="ze")
                            nc.vector.tensor_scalar_mul(z[:], accs[wl][:],
                                                        dinvsb[:, w:w + 1])
                            z2 = sbE.tile([P, H2], F32, tag="z2e")
                            nc.vector.tensor_tensor(z2[:], z[:], b2sb[:],
                                                    op=mybir.AluOpType.add)
                            a2 = sbE.tile([P, H2], BF16, tag="a2")
                            nc.scalar.activation(a2[:], z2[:],
                                                 mybir.ActivationFunctionType.Relu)
                            a2tp = psL.tile([P, P], BF16, tag="a2tp")
                            nc.tensor.transpose(a2tp[:], a2[:], idsb[:])
                            a2t = sbE.tile([P, P], BF16, tag="a2t")
                            nc.vector.tensor_copy(a2t[:], a2tp[:])
                            lg = psL.tile([P, C], F32, tag="lg")
                            nc.tensor.matmul(lg[:], a2t[:], wlsb[:],
                                             start=True, stop=True)
                            nc.vector.tensor_tensor(lgst[:, w * C:(w + 1) * C],
                                                    lg[:], blsb[:],
                                                    op=mybir.AluOpType.add)
                            e = sbE.tile([P, C], F32, tag="e")
                            nc.scalar.activation(e[:], lgst[:, w * C:(w + 1) * C],
                                                 mybir.ActivationFunctionType.Exp,
                                                 accum_out=sst[:, w:w + 1])
                    lns = cpool.tile([P, W], F32)
                    nc.scalar.activation(lns[:], sst[:],
                                         mybir.ActivationFunctionType.Ln)
                    for w in range(W):
                        nc.vector.tensor_scalar(
                            outsb[:, w * C:(w + 1) * C], lgst[:, w * C:(w + 1) * C],
                            lns[:, w:w + 1], None, op0=mybir.AluOpType.subtract)
                    nc.sync.dma_start(outst[:], outsb[:])
            else:
                # debug variants: dummy output proving the kept phases ran
                nc.vector.memset(outsb[:], 0.0)
                if upto != "noop":
                    probe_src = ag1_out[0] if upto == "A" else ag2_out[0]
                    probe = cpool.tile([P, C], BF16)
                    nc.sync.dma_start(probe[:], probe_src[:P, :C])
                    nc.vector.tensor_copy(outsb[:, :C], probe[:])
                nc.sync.dma_start(outst[:], outsb[:])

    nc.compile()
    return nc

# ---------------------------------------------------------------- entry point
_CACHE = {}


def _get_compiled(cfg, key, meta):
    if key not in _CACHE:
        nc = build_kernel(cfg, meta)
        nc.m = get_hw_module(nc.m)
        _CACHE[key] = nc
    return _CACHE[key]


def run(cfg, inputs):
    in_maps, meta = preprocess(cfg, **inputs)
    key = (cfg.N, cfg.F_IN, meta["TT"])
    nc = _get_compiled(cfg, key, meta)
    res = bass_utils.run_bass_kernel_spmd(
        nc, in_maps, core_ids=list(range(cfg.ncores)))
    out = assemble_output(cfg, meta, res.results)
    return out, res


class _TimedRunner:
    """PJRT runner mirroring bass2jax.run_bass_via_pjrt's multi-core branch,
    but with a cached jit and device-resident inputs for repeatable timing."""

    def __init__(self, nc, n_cores):
        import jax
        import concourse.mybir as mb
        from concourse import bass2jax
        from jax.sharding import Mesh, PartitionSpec, NamedSharding
        from jax.experimental.shard_map import shard_map

        bass2jax.install_neuronx_cc_hook()
        partition_name = (nc.partition_id_tensor.name
                          if nc.partition_id_tensor else None)
        in_names, out_names, out_avals, zero_shapes = [], [], [], []
        for alloc in nc.m.functions[0].allocations:
            if not isinstance(alloc, mb.MemoryLocationSet):
                continue
            name = alloc.memorylocations[0].name
            if alloc.kind == "ExternalInput":
                if name != partition_name:
                    in_names.append(name)
            elif alloc.kind == "ExternalOutput":
                out_names.append(name)
                shape = tuple(alloc.tensor_shape)
                dtype = mb.dt.np(alloc.dtype)
                out_avals.append(jax.core.ShapedArray(shape, dtype))
                zero_shapes.append((shape, dtype))
        n_params = len(in_names)
        all_in_names = list(in_names) + list(out_names)
        if partition_name is not None:
            all_in_names.append(partition_name)
        donate = tuple(range(n_params, n_params + len(out_names)))

        def _exec(*operands_in):
            operands = list(operands_in)
            if partition_name is not None:
                operands.append(bass2jax.partition_id_tensor())
            outs = bass2jax._bass_exec_p.bind(
                *operands,
                out_avals=tuple(out_avals),
                in_names=tuple(all_in_names),
                out_names=tuple(out_names),
                lowering_input_output_aliases=(),
                sim_require_finite=True,
                sim_require_nnan=True,
                nc=nc,
            )
            return tuple(outs)

        def _body(*args):
            return _exec(*args)

        def _body_k(k):
            # k back-to-back kernel executions inside ONE XLA program:
            # call i+1 consumes call i's outputs as its output-buffer
            # operands, so the chain cannot be CSE'd and the device runs
            # them serially, while the host pays a single dispatch.
            def f(*args):
                ins = args[:n_params]
                outs = list(args[n_params:])
                for _ in range(k):
                    outs = list(_exec(*ins, *outs))
                return tuple(outs)
            return f

        self._body_k = _body_k
        self.n_params = n_params

        devices = jax.devices()[:n_cores]
        mesh = Mesh(np.asarray(devices), ("core",))
        in_specs = (PartitionSpec("core"),) * (n_params + len(out_names))
        out_specs = (PartitionSpec("core"),) * len(out_names)
        # No donation: output buffers are not aliased, so back-to-back
        # executions have no host-side data dependency and submissions can
        # pipeline ahead of device execution (the device serializes them).
        del donate
        self.fn = jax.jit(
            shard_map(_body, mesh=mesh, in_specs=in_specs,
                      out_specs=out_specs, check_rep=False),
            keep_unused=True)
        self.jax = jax
        self.mesh = mesh
        self.sharding = NamedSharding(mesh, PartitionSpec("core"))
        self.in_names = in_names
        self.out_names = out_names
        self.zero_shapes = zero_shapes
        self.n_cores = n_cores
        self.dev_inputs = None
        self._shard_map = shard_map
        self._in_specs = in_specs
        self._out_specs = out_specs
        self._fnk = {}

    def stage_inputs(self, in_maps):
        concat_in = [
            np.concatenate([np.asarray(in_maps[c][n])
                            for c in range(self.n_cores)], axis=0)
            for n in self.in_names
        ]
        self.dev_inputs = [self.jax.device_put(a, self.sharding)
                           for a in concat_in]
        for a in self.dev_inputs:
            a.block_until_ready()

    def fresh_outs(self):
        zeros = [np.zeros((self.n_cores * s[0], *s[1:]), d)
                 for s, d in self.zero_shapes]
        dz = [self.jax.device_put(z, self.sharding) for z in zeros]
        for z in dz:
            z.block_until_ready()
        return dz

    def exec_once(self):
        import time
        dz = self.fresh_outs()
        t0 = time.perf_counter()
        outs = self.fn(*self.dev_inputs, *dz)
        for o in outs:
            o.block_until_ready()
        t1 = time.perf_counter()
        return outs, t1 - t0

    def exec_chained(self, n):
        """Run n back-to-back executions, timing the whole pipeline and
        dividing by n. Executions are independent (same device-resident
        inputs, no donation), amortizing the host/tunnel dispatch overhead
        that a single blocking execution would include."""
        import time
        dz = self.fresh_outs()
        cur = None
        t0 = time.perf_counter()
        for _ in range(n):
            cur = list(self.fn(*self.dev_inputs, *dz))
        for o in cur:
            o.block_until_ready()
        t1 = time.perf_counter()
        return cur, (t1 - t0) / n

    def results(self, outs):
        res = []
        for c in range(self.n_cores):
            m = {}
            for i, n in enumerate(self.out_names):
                full = np.asarray(outs[i])
                per = full.reshape(self.n_cores, -1, *full.shape[1:])[c]
                m[n] = per
            res.append(m)
        return res


def run_timed(cfg, inputs, iters=3, chain=0):
    in_maps, meta = preprocess(cfg, **inputs)
    key = (cfg.N, cfg.F_IN, meta["TT"])
    nc = _get_compiled(cfg, key, meta)
    rkey = ("runner",) + key
    if rkey not in _CACHE:
        _CACHE[rkey] = _TimedRunner(nc, cfg.ncores)
    runner = _CACHE[rkey]
    runner.stage_inputs(in_maps)
    times = []
    outs = None
    if chain:
        runner.exec_once()  # warmup
        for _ in range(iters):
            outs, dt = runner.exec_chained(chain)
            times.append(dt)
    else:
        for _ in range(iters):
            outs, dt = runner.exec_once()
            times.append(dt)
    results = runner.results(outs)
    out = assemble_output(cfg, meta, results)
    return out, times


def kernel(x, edge_index, W1, b1, W2, b2, Wl, bl):
    out, _ = run(FULL, dict(x=x, edge_index=edge_index, W1=W1, b1=b1,
                            W2=W2, b2=b2, Wl=Wl, bl=bl))
    return out
